# revision 11
# baseline (speedup 1.0000x reference)
"""Trainium2 Bass kernel for nn_CompLayer_37512244363763 (GNN message passing).

Strategy: dst-range sharding over 8 cores (no collectives). Each core owns
nodes [c*5000, (c+1)*5000). Host pre-sorts edges by (dst-block, src-half),
pads each (block, src-half) section to 128-edge chunks with shared-across-core
capacities so one SPMD program serves all cores.

v3 design:
- h[src] / rel[rel_id] fetched with plain (edge-major) f32 dma_gathers
  (512B rows) spread over 4 SWDGE queues (~4x single-queue rate).
- h[dst] gather ELIMINATED: block features SBUF-resident as hbT [128d,
  5000n] fp16. comp = hs*rl computed in fp16 edge-major; per 128-edge
  chunk a PE transpose makes compT, then S = compT^T @ hbT_b scores the
  chunk against all 125 block nodes. exp(S - 30) on the Scalar engine
  (shift-invariant softmax, avoids overflow); one-hot A0 (built batched
  per super on DVE) masks it: A = A0 * exp(S).
- Accumulation matmul psum[n,129] += A^T @ [comp | 1] (bf16 x fp16).
- Epilogue per block in fp32: den/recip, scale-copy, transpose, GEMM
  with W, tanh, store transposed (host untransposes).
"""
import numpy as np
import ml_dtypes

import concourse.bacc as bacc
import concourse.bass as bass
import concourse.tile as tile
import concourse.mybir as mybir
from concourse.bass_utils import run_bass_kernel_spmd
from concourse.masks import make_identity

f32 = mybir.dt.float32
f16 = mybir.dt.float16
bf16 = mybir.dt.bfloat16
i16 = mybir.dt.int16
nbf16 = ml_dtypes.bfloat16

N = 40000
E = 640000
D = 128
R2 = 474
NCORES = 8
NS = N // NCORES          # nodes per core
BLK = 125                 # nodes per PSUM block
NBLK = NS // BLK          # blocks per core
P = 128                   # edges per chunk
SB = 2                    # blocks per gather super
SPLIT = 32768             # int16 index limit for src gathers
GRP = 4                   # chunks per PSUM score group
SHIFT = 30.0              # softmax logit shift (overflow guard)
NQ = 4                    # SWDGE queues


def _ceil128(x):
    return ((x + 127) // 128) * 128


def build_layout(ent, rel_emb, neigh_w, src, dst, rel_id, split=SPLIT,
                 n=N, d=D, ncores=NCORES, blk=BLK):
    """Host-side shard + sort + pad. Returns (meta, in_maps)."""
    ns = n // ncores
    nblk = ns // blk
    e = src.shape[0]

    gb = dst.astype(np.int64) // blk                   # global block id
    half = (src >= split).astype(np.int64)
    key = gb * 2 + half
    order = np.argsort(key, kind="stable")
    nsec_g = ncores * nblk * 2
    cnt = np.bincount(key, minlength=nsec_g)
    Lc = cnt[0::2].reshape(ncores, nblk)
    Hc = cnt[1::2].reshape(ncores, nblk)
    capL = _ceil128(Lc.max(axis=0))
    capH = _ceil128(Hc.max(axis=0))
    empty = (capL + capH) == 0
    capL[empty] = 128
    n_chunks = (capL + capH) // 128                    # per in-core block
    c_total = int(n_chunks.sum())
    totslot = 128 * c_total

    blockstart = np.zeros(nblk + 1, np.int64)
    blockstart[1:] = np.cumsum(capL + capH)
    secstart = np.zeros(nblk * 2, np.int64)
    secstart[0::2] = blockstart[:-1]
    secstart[1::2] = blockstart[:-1] + capL

    gfirst = np.zeros(nsec_g, np.int64)
    gfirst[1:] = np.cumsum(cnt)[:-1]
    ranks = np.arange(e, dtype=np.int64) - np.repeat(gfirst, cnt)
    key_sorted = key[order]
    sec_local = (key_sorted % (nblk * 2))
    slot_sorted = secstart[sec_local] + ranks
    core_sorted = key_sorted // (nblk * 2)

    src_s = src[order].astype(np.int64)
    dst_s = dst[order].astype(np.int64)
    rel_s = rel_id[order].astype(np.int64)

    def wrap16(flat):
        w = flat.reshape(totslot // 16, 16).T          # [16, cols]
        return np.tile(w, (8, 1)).copy()               # [128, cols]

    iota = np.broadcast_to(np.arange(blk, dtype=np.float32),
                           (P, blk)).astype(nbf16).copy()
    ent32 = ent.astype(np.float32)
    rel32 = rel_emb.astype(np.float32)

    in_maps = []
    for c in range(ncores):
        m = core_sorted == c
        slots = slot_sorted[m]
        srcf = np.zeros(totslot, np.int16)
        relf = np.zeros(totslot, np.int16)
        dlf = np.full(totslot, -1.0, np.float32)
        sc = src_s[m]
        srcf[slots] = np.where(sc >= split, sc - split, sc).astype(np.int16)
        relf[slots] = rel_s[m].astype(np.int16)
        dloc = dst_s[m] - c * ns
        dlf[slots] = (dloc - (dloc // blk) * blk).astype(np.float32)

        in_maps.append({
            "ent": ent32,
            "rel": rel32,
            "hbt": np.ascontiguousarray(
                ent32[c * ns:(c + 1) * ns].T).astype(np.float16),
            "w": neigh_w.astype(np.float32),
            "iota": iota,
            "srcidx": wrap16(srcf),
            "relidx": wrap16(relf),
            "dstlf": dlf.reshape(c_total, 128).T.astype(nbf16).copy(),
        })

    meta = dict(capL=capL, capH=capH, n_chunks=n_chunks,
                blockstart=blockstart, c_total=c_total, totslot=totslot,
                nblk=nblk, ns=ns, n=n, d=d, split=split)
    return meta, in_maps


DBG = False


def build_program(meta, repeat=1):
    n, d, ns, nblk = meta["n"], meta["d"], meta["ns"], meta["nblk"]
    split = meta["split"]
    capL, capH, n_chunks = meta["capL"], meta["capH"], meta["n_chunks"]
    blockstart = meta["blockstart"]
    c_total, totslot = meta["c_total"], meta["totslot"]
    blk = BLK

    nc = bacc.Bacc("TRN2", target_bir_lowering=False, debug=False,
                   num_swdge_queues=NQ)
    ent_d = nc.dram_tensor("ent", [n, d], f32, kind="ExternalInput")
    rel_d = nc.dram_tensor("rel", [R2, d], f32, kind="ExternalInput")
    hbt_d = nc.dram_tensor("hbt", [d, ns], f16, kind="ExternalInput")
    w_d = nc.dram_tensor("w", [d, d], f32, kind="ExternalInput")
    iota_d = nc.dram_tensor("iota", [P, blk], bf16, kind="ExternalInput")
    srcidx_d = nc.dram_tensor("srcidx", [P, totslot // 16], i16,
                              kind="ExternalInput")
    relidx_d = nc.dram_tensor("relidx", [P, totslot // 16], i16,
                              kind="ExternalInput")
    dstlf_d = nc.dram_tensor("dstlf", [P, c_total], bf16,
                             kind="ExternalInput")
    outT_d = nc.dram_tensor("outT", [d, ns], f32, kind="ExternalOutput")
    if DBG:
        nidx0 = int(blockstart[SB]) - int(blockstart[0])
        nch0 = nidx0 // 128
        dbg_compp = nc.dram_tensor("dbg_compp", [P, nch0 * (d + 1)], f16,
                                   kind="ExternalOutput")
        dbg_A = nc.dram_tensor("dbg_A", [P, nch0 * BLK], bf16,
                               kind="ExternalOutput")
        dbg_compT = nc.dram_tensor("dbg_compT", [P, nch0 * d], f16,
                                   kind="ExternalOutput")

    nsup = nblk // SB
    qctr = [0]

    def nextq():
        q = qctr[0] % NQ
        qctr[0] += 1
        return q

    with tile.TileContext(nc) as tc:
        with (
            tc.tile_pool(name="const", bufs=1) as cp,
            tc.tile_pool(name="sup", bufs=2) as supp,
            tc.tile_pool(name="gath", bufs=2) as gp,
            tc.tile_pool(name="small", bufs=4) as smp,
            tc.tile_pool(name="epi", bufs=2) as epp,
            tc.tile_pool(name="psS", bufs=2, space="PSUM") as psS,
            tc.tile_pool(name="psT", bufs=2, space="PSUM") as psT,
            tc.tile_pool(name="psA", bufs=2, space="PSUM") as psA,
            tc.tile_pool(name="psN", bufs=1, space="PSUM") as psN,
            tc.tile_pool(name="psO", bufs=1, space="PSUM") as psO,
        ):
            iota_t = cp.tile([P, blk], bf16)
            nc.sync.dma_start(out=iota_t[:], in_=iota_d[:])
            w_t = cp.tile([d, d], f32)
            nc.sync.dma_start(out=w_t[:], in_=w_d[:])
            hbt_t = cp.tile([d, ns], f16)
            nc.sync.dma_start(out=hbt_t[:], in_=hbt_d[:])
            identh = cp.tile([P, P], f16)
            make_identity(nc, identh[:])
            identf = cp.tile([P, P], f32)
            make_identity(nc, identf[:])
            shiftc = cp.tile([P, 1], f32)
            nc.vector.memset(shiftc[:], -SHIFT)

            def body(_iv=None):
                for s in range(nsup):
                    b0 = s * SB
                    blks = [b for b in range(b0, b0 + SB)]
                    ss0 = int(blockstart[b0])
                    ss1 = int(blockstart[b0 + SB])
                    nidx = ss1 - ss0
                    nch = nidx // 128
                    c0 = ss0 // 128
                    col0, col1 = ss0 // 16, ss1 // 16

                    srcw = supp.tile([P, col1 - col0], i16, tag="srcw")
                    relw = supp.tile([P, col1 - col0], i16, tag="relw")
                    dlfw = supp.tile([P, nch], bf16, tag="dlfw")
                    nc.sync.dma_start(out=srcw[:], in_=srcidx_d[:, col0:col1])
                    nc.sync.dma_start(out=relw[:], in_=relidx_d[:, col0:col1])
                    nc.sync.dma_start(out=dlfw[:], in_=dstlf_d[:, c0:c0 + nch])

                    # ---- edge-major f32 gathers: hs per (block, half), rl per block
                    hs_tiles = {}
                    rl_tiles = {}
                    for i, b in enumerate(blks):
                        for hi, cap, base in ((0, int(capL[b]), 0),
                                              (1, int(capH[b]), split)):
                            if cap == 0:
                                continue
                            sec0 = int(blockstart[b]) + (int(capL[b]) if hi else 0)
                            lo = sec0 - ss0
                            t = gp.tile([P, cap], f32, tag=f"hs{i}{hi}")
                            nc.gpsimd.dma_gather(
                                out_ap=t[:].rearrange("p (c x) -> p c x", x=d),
                                in_ap=ent_d[base:, :] if base else ent_d[:],
                                idxs_ap=srcw[:, lo // 16:(lo + cap) // 16],
                                num_idxs=cap,
                                num_idxs_reg=cap,
                                elem_size=d,
                                single_packet=False,
                                queue_num=nextq(),
                            )
                            hs_tiles[(i, hi)] = (t, lo, cap)
                        capb = int(capL[b]) + int(capH[b])
                        lob = int(blockstart[b]) - ss0
                        rt = gp.tile([P, capb], f32, tag=f"rl{i}")
                        nc.gpsimd.dma_gather(
                            out_ap=rt[:].rearrange("p (c x) -> p c x", x=d),
                            in_ap=rel_d[:],
                            idxs_ap=relw[:, lob // 16:(lob + capb) // 16],
                            num_idxs=capb,
                            num_idxs_reg=capb,
                            elem_size=d,
                            single_packet=False,
                            queue_num=nextq(),
                        )
                        rl_tiles[i] = (rt, lob, capb)

                    # ---- compp[e, c, 0:128] = hs*rl (fp16); col 128 = 1
                    compp = supp.tile([P, nch * (d + 1)], f16, tag="compp")
                    compp3 = compp[:].rearrange("p (c x) -> p c x", x=d + 1)
                    nc.vector.memset(compp3[:, :, d:d + 1], 1.0)
                    for i, b in enumerate(blks):
                        rt, lob, capb = rl_tiles[i]
                        rt3 = rt[:].rearrange("p (c x) -> p c x", x=d)
                        for hi in (0, 1):
                            if (i, hi) not in hs_tiles:
                                continue
                            t, lo, cap = hs_tiles[(i, hi)]
                            t3 = t[:].rearrange("p (c x) -> p c x", x=d)
                            nc.vector.tensor_tensor(
                                out=compp3[:, lo // 128:(lo + cap) // 128, :d],
                                in0=t3,
                                in1=rt3[:, (lo - lob) // 128:
                                        (lo - lob + cap) // 128, :],
                                op=mybir.AluOpType.mult,
                            )

                    # ---- one-hot A0[e, c, n] = (iota[n] == dlf[e, c])
                    A0 = supp.tile([P, nch * blk], bf16, tag="A0")
                    A03 = A0[:].rearrange("p (c x) -> p c x", x=blk)
                    nc.vector.tensor_tensor(
                        out=A03,
                        in0=iota_t[:].unsqueeze(1).broadcast_to((P, nch, blk)),
                        in1=dlfw[:].unsqueeze(2).broadcast_to((P, nch, blk)),
                        op=mybir.AluOpType.is_equal,
                    )

                    compT = supp.tile([P, nch * d], f16, tag="compT")

                    ci = c0
                    for i, b in enumerate(blks):
                        nch_b = int(n_chunks[b])
                        base_c = ci - c0
                        # pass 1: scores -> A
                        for g0 in range(0, nch_b, GRP):
                            gs = min(GRP, nch_b - g0)
                            T_ps = psT.tile([P, GRP * d], f16, tag="T")
                            for k in range(gs):
                                ch = base_c + g0 + k
                                nc.tensor.transpose(
                                    out=T_ps[:, k * d:(k + 1) * d],
                                    in_=compp3[:, ch, :d],
                                    identity=identh[:])
                            nc.scalar.activation(
                                out=compT[:, (base_c + g0) * d:
                                          (base_c + g0 + gs) * d],
                                in_=T_ps[:, :gs * d],
                                func=mybir.ActivationFunctionType.Copy)
                            S_ps = psS.tile([P, GRP * blk], f32, tag="S")
                            for k in range(gs):
                                ch = base_c + g0 + k
                                nc.tensor.matmul(
                                    out=S_ps[:, k * blk:(k + 1) * blk],
                                    lhsT=compT[:, ch * d:(ch + 1) * d],
                                    rhs=hbt_t[:, b * blk:(b + 1) * blk],
                                    start=True, stop=True)
                            expS = smp.tile([P, GRP * blk], bf16, tag="expS")
                            nc.scalar.activation(
                                out=expS[:, :gs * blk],
                                in_=S_ps[:, :gs * blk],
                                func=mybir.ActivationFunctionType.Exp,
                                bias=shiftc[:])
                            A0g = A03[:, base_c + g0:base_c + g0 + gs, :]
                            nc.vector.tensor_tensor(
                                out=A0g, in0=A0g,
                                in1=expS[:, :gs * blk]
                                    .rearrange("p (c x) -> p c x", x=blk),
                                op=mybir.AluOpType.mult,
                            )
                        # pass 2: accumulation
                        psum_b = psA.tile([blk, d + 1], f32, tag="acc")
                        for kk in range(nch_b):
                            ch = base_c + kk
                            nc.tensor.matmul(
                                out=psum_b[:],
                                lhsT=A03[:, ch, :],
                                rhs=compp3[:, ch, :],
                                start=(kk == 0), stop=(kk == nch_b - 1))
                        # block epilogue (fp32)
                        den = epp.tile([blk, 1], f32, tag="den")
                        nc.vector.tensor_scalar_add(
                            out=den[:], in0=psum_b[:, d:d + 1], scalar1=1e-37)
                        rinv = epp.tile([blk, 1], f32, tag="rinv")
                        nc.vector.reciprocal(out=rinv[:], in_=den[:])
                        nb = epp.tile([blk, d], f32, tag="nb")
                        nc.scalar.activation(
                            out=nb[:], in_=psum_b[:, :d],
                            func=mybir.ActivationFunctionType.Copy,
                            scale=rinv[:])
                        nT_ps = psN.tile([d, blk], f32, tag="nT")
                        nc.tensor.transpose(
                            out=nT_ps[:], in_=nb[:],
                            identity=identf[:blk, :blk])
                        nT = epp.tile([d, blk], f32, tag="nTs")
                        nc.vector.tensor_copy(out=nT[:], in_=nT_ps[:])
                        oT_ps = psO.tile([d, blk], f32, tag="oT")
                        nc.tensor.matmul(out=oT_ps[:], lhsT=w_t[:], rhs=nT[:],
                                         start=True, stop=True)
                        ob = epp.tile([d, blk], f32, tag="ob")
                        nc.scalar.activation(
                            out=ob[:], in_=oT_ps[:],
                            func=mybir.ActivationFunctionType.Tanh)
                        nc.sync.dma_start(
                            out=outT_d[:, b * blk:(b + 1) * blk], in_=ob[:])
                        ci += nch_b

                    if DBG and s == 0:
                        nc.sync.dma_start(out=dbg_compp[:], in_=compp[:])
                        nc.sync.dma_start(out=dbg_A[:], in_=A0[:])
                        nc.sync.dma_start(out=dbg_compT[:], in_=compT[:])

            if repeat == 1:
                body()
            else:
                with tc.For_i(0, repeat, 1) as iv:
                    body(iv)

    nc.compile()
    return nc


_CACHE = {}


def _get_compiled(meta):
    key = (meta["c_total"], tuple(meta["n_chunks"]), tuple(meta["capL"]),
           meta["n"], meta["d"])
    if key not in _CACHE:
        _CACHE[key] = build_program(meta)
    return _CACHE[key]


def kernel(ent_emb, rel_emb, neigh_w, node_id, src, dst, rel_id):
    ent_emb = np.asarray(ent_emb, dtype=np.float32)
    rel_emb = np.asarray(rel_emb, dtype=np.float32)
    neigh_w = np.asarray(neigh_w, dtype=np.float32)
    node_id = np.asarray(node_id, dtype=np.int32)
    src = np.asarray(src, dtype=np.int32)
    dst = np.asarray(dst, dtype=np.int32)
    rel_id = np.asarray(rel_id, dtype=np.int32)

    ent = np.ascontiguousarray(ent_emb[node_id])   # node features (arange id)
    meta, in_maps = build_layout(ent, rel_emb, neigh_w, src, dst, rel_id)
    nc = _get_compiled(meta)
    res = run_bass_kernel_spmd(nc, in_maps, core_ids=list(range(NCORES)))
    out = np.concatenate([res.results[c]["outT"].T for c in range(NCORES)],
                         axis=0)
    return out.astype(np.float32)
